# revision 1
# baseline (speedup 1.0000x reference)
"""Llama4 MoE experts + shared LoRA, expert-parallel on 8 TRN2 NeuronCores.

Per-core (expert e): x[1024,1024] @ W_gu[1024,4096] (+ rank-8 LoRA) -> SwiGLU
-> h[1024,2048] @ W_dn[2048,1024] (+ rank-8 LoRA) -> out[1024,1024].

Layout strategy: keep the intermediate transposed. gate_up^T tiles
[128 F-part, 512 T-free] come straight out of PSUM with H contracted on the
partition dim; SwiGLU runs on the transposed tiles, producing hiddenT with I on
partitions -- exactly the contraction layout the down matmul needs, whose
output is then the natural [T, H]. Only x (4 MiB) is PE-transposed on chip.

All matmuls run in float32r (fp32 with 11-bit RNE mantissa, 1 cycle/row for
free-dim >= 256 -- bf16 speed at ~1.5e-4 relative error).
"""
import sys

sys.path.insert(0, "/opt/trn_rl_repo")

import numpy as np

import concourse.bacc as bacc
import concourse.bass as bass
import concourse.mybir as mybir
import concourse.tile as tile
from concourse.bass_utils import run_bass_kernel_spmd
from concourse.masks import make_identity

E = 8           # experts == cores
T = 1024        # tokens per expert
H = 1024        # hidden
I = 2048        # intermediate
F2 = 2 * I      # gate+up
R = 8           # lora rank
SCALING = 2.0   # lora_alpha / rank
P = 128         # partitions
NFREE = 512     # moving free-dim per matmul (one PSUM bank of fp32)
KH = H // P     # 8 k-tiles over H
KI = I // P     # 16 k-tiles over I
NT = T // NFREE     # 2 T-chunks
NH = H // NFREE     # 2 H-chunks
NFP = I // P        # 16 F-pair tiles (gate i pairs with up i+16)

F32 = mybir.dt.float32
F32R = mybir.dt.float32r


def build_kernel():
    nc = bacc.Bacc("TRN2", target_bir_lowering=False, debug=False)

    x_d = nc.dram_tensor("x", [T, H], F32, kind="ExternalInput")
    wgu_d = nc.dram_tensor("w_gu", [H, F2], F32R, kind="ExternalInput")
    wdn_d = nc.dram_tensor("w_dn", [I, H], F32R, kind="ExternalInput")
    agu_d = nc.dram_tensor("a_gu", [R, H], F32, kind="ExternalInput")
    bgu_d = nc.dram_tensor("b_gu", [F2, R], F32, kind="ExternalInput")
    adn_d = nc.dram_tensor("a_dn", [R, I], F32, kind="ExternalInput")
    bdn_d = nc.dram_tensor("b_dn", [H, R], F32, kind="ExternalInput")
    out_d = nc.dram_tensor("out", [T, H], F32, kind="ExternalOutput")

    with tile.TileContext(nc) as tc:
        with (
            tc.tile_pool(name="const", bufs=1) as const_pool,
            tc.tile_pool(name="xT", bufs=1) as xT_pool,
            tc.tile_pool(name="hT", bufs=1) as hT_pool,
            tc.tile_pool(name="smalls", bufs=1) as small_pool,
            tc.tile_pool(name="xnat", bufs=2) as xnat_pool,
            tc.tile_pool(name="wgu", bufs=4) as wgu_pool,
            tc.tile_pool(name="wdn", bufs=4) as wdn_pool,
            tc.tile_pool(name="silu", bufs=3) as silu_pool,
            tc.tile_pool(name="outs", bufs=3) as out_pool,
            tc.tile_pool(name="ps_tr", bufs=2, space="PSUM") as ps_tr,
            tc.tile_pool(name="ps_mm", bufs=6, space="PSUM") as ps_mm,
        ):
            ident = const_pool.tile([P, P], F32)
            make_identity(nc, ident[:])

            # ---- phase A: transpose x into xT[k] = [128 H, 1024 T] (f32r) ----
            xT = [xT_pool.tile([P, T], F32R, tag=f"xT{k}", name=f"xT{k}") for k in range(KH)]
            for b in range(T // P):
                xb = xnat_pool.tile([P, H], F32)
                nc.sync.dma_start(xb[:], x_d[P * b:P * (b + 1), :])
                for k in range(KH):
                    ps = ps_tr.tile([P, P], F32, tag="tr")
                    nc.tensor.transpose(ps[:], xb[:, P * k:P * (k + 1)], ident[:])
                    nc.vector.tensor_copy(xT[k][:, P * b:P * (b + 1)], ps[:])

            # ---- small lora tensors: transpose to matmul layouts (f32r) ----
            # A_guT[k]: [128 H, 8 R]
            agu_nat = small_pool.tile([R, H], F32, tag="agu_nat")
            nc.sync.dma_start(agu_nat[:], agu_d[:])
            aguT = [small_pool.tile([P, R], F32R, tag=f"aguT{k}", name=f"aguT{k}") for k in range(KH)]
            for k in range(KH):
                ps = ps_tr.tile([P, R], F32, tag="tr")
                nc.tensor.transpose(ps[:], agu_nat[:, P * k:P * (k + 1)],
                                    ident[:R, :R])
                nc.vector.tensor_copy(aguT[k][:], ps[:])
            # A_dnT[k]: [128 I, 8 R]
            adn_nat = small_pool.tile([R, I], F32, tag="adn_nat")
            nc.sync.dma_start(adn_nat[:], adn_d[:])
            adnT = [small_pool.tile([P, R], F32R, tag=f"adnT{k}", name=f"adnT{k}") for k in range(KI)]
            for k in range(KI):
                ps = ps_tr.tile([P, R], F32, tag="tr")
                nc.tensor.transpose(ps[:], adn_nat[:, P * k:P * (k + 1)],
                                    ident[:R, :R])
                nc.vector.tensor_copy(adnT[k][:], ps[:])
            # B_guT: [8 R, 4096 F]
            bgu_nat = small_pool.tile([P, F2 // P, R], F32, tag="bgu_nat")
            nc.sync.dma_start(bgu_nat[:],
                              bgu_d[:].rearrange("(bo bi) r -> bi bo r", bi=P))
            bguT = small_pool.tile([R, F2], F32R, tag="bguT")
            for bo in range(F2 // P):
                ps = ps_tr.tile([R, P], F32, tag="tr")
                nc.tensor.transpose(ps[:], bgu_nat[:, bo, :], ident[:])
                nc.vector.tensor_copy(bguT[:, P * bo:P * (bo + 1)], ps[:])
            # B_dnT: [8 R, 1024 H]
            bdn_nat = small_pool.tile([P, H // P, R], F32, tag="bdn_nat")
            nc.sync.dma_start(bdn_nat[:],
                              bdn_d[:].rearrange("(bo bi) r -> bi bo r", bi=P))
            bdnT = small_pool.tile([R, H], F32R, tag="bdnT")
            for bo in range(H // P):
                ps = ps_tr.tile([R, P], F32, tag="tr")
                nc.tensor.transpose(ps[:], bdn_nat[:, bo, :], ident[:])
                nc.vector.tensor_copy(bdnT[:, P * bo:P * (bo + 1)], ps[:])

            # ---- r1T = SCALING * (A_gu @ x^T): [8 R, 1024 T] ----
            r1T = small_pool.tile([R, T], F32R, tag="r1T")
            for t in range(NT):
                ps = ps_mm.tile([R, NFREE], F32, tag="mm")
                for k in range(KH):
                    nc.tensor.matmul(ps[:], aguT[k][:],
                                     xT[k][:, NFREE * t:NFREE * (t + 1)],
                                     start=(k == 0), stop=(k == KH - 1))
                nc.vector.tensor_scalar_mul(
                    r1T[:, NFREE * t:NFREE * (t + 1)], ps[:], SCALING)

            # ---- phase B: gate_up^T + SwiGLU -> hiddenT[k] [128 I, 1024 T] ----
            hT = [hT_pool.tile([P, T], F32R, tag=f"hT{k}", name=f"hT{k}") for k in range(KI)]
            for i in range(NFP):
                wg = wgu_pool.tile([P, KH, P], F32R, tag="wgu")
                wu = wgu_pool.tile([P, KH, P], F32R, tag="wgu")
                fg, fu = P * i, P * (i + NFP)
                nc.sync.dma_start(
                    wg[:], wgu_d[:, fg:fg + P].rearrange("(ko ki) f -> ki ko f", ki=P))
                nc.sync.dma_start(
                    wu[:], wgu_d[:, fu:fu + P].rearrange("(ko ki) f -> ki ko f", ki=P))
                for t in range(NT):
                    ts = slice(NFREE * t, NFREE * (t + 1))
                    psg = ps_mm.tile([P, NFREE], F32, tag="mm")
                    psu = ps_mm.tile([P, NFREE], F32, tag="mm")
                    for k in range(KH):
                        nc.tensor.matmul(psg[:], wg[:, k, :], xT[k][:, ts],
                                         start=(k == 0), stop=False)
                    nc.tensor.matmul(psg[:], bguT[:, fg:fg + P], r1T[:, ts],
                                     start=False, stop=True)
                    for k in range(KH):
                        nc.tensor.matmul(psu[:], wu[:, k, :], xT[k][:, ts],
                                         start=(k == 0), stop=False)
                    nc.tensor.matmul(psu[:], bguT[:, fu:fu + P], r1T[:, ts],
                                     start=False, stop=True)
                    sg = silu_pool.tile([P, NFREE], F32, tag="silu")
                    nc.scalar.activation(sg[:], psg[:],
                                         mybir.ActivationFunctionType.Silu)
                    nc.vector.tensor_mul(hT[i][:, ts], sg[:], psu[:])

            # ---- r2T = SCALING * (A_dn @ hidden^T): [8 R, 1024 T] ----
            r2T = small_pool.tile([R, T], F32R, tag="r2T")
            for t in range(NT):
                ps = ps_mm.tile([R, NFREE], F32, tag="mm")
                for k in range(KI):
                    nc.tensor.matmul(ps[:], adnT[k][:],
                                     hT[k][:, NFREE * t:NFREE * (t + 1)],
                                     start=(k == 0), stop=(k == KI - 1))
                nc.vector.tensor_scalar_mul(
                    r2T[:, NFREE * t:NFREE * (t + 1)], ps[:], SCALING)

            # ---- phase D: out[T, H] = hidden @ W_dn + lora ----
            for h in range(NH):
                hs = slice(NFREE * h, NFREE * (h + 1))
                for grp in range(2):
                    pos = []
                    for jj in range(4):
                        pos.append(ps_mm.tile([P, NFREE], F32, tag="mm", name=f"po{h}_{grp}_{jj}"))
                    for k in range(KI):
                        wd = wdn_pool.tile([P, NFREE], F32R, tag="wdn")
                        nc.sync.dma_start(wd[:], wdn_d[P * k:P * (k + 1), hs])
                        for jj in range(4):
                            j = 4 * grp + jj
                            nc.tensor.matmul(pos[jj][:],
                                             hT[k][:, P * j:P * (j + 1)], wd[:],
                                             start=(k == 0), stop=False)
                    for jj in range(4):
                        j = 4 * grp + jj
                        nc.tensor.matmul(pos[jj][:], r2T[:, P * j:P * (j + 1)],
                                         bdnT[:, hs], start=False, stop=True)
                        ot = out_pool.tile([P, NFREE], F32, tag="outs")
                        nc.scalar.activation(ot[:], pos[jj][:],
                                             mybir.ActivationFunctionType.Copy)
                        nc.sync.dma_start(out_d[P * j:P * (j + 1), hs], ot[:])

    nc.finalize()
    return nc


_NC_CACHE = None


def _get_nc():
    global _NC_CACHE
    if _NC_CACHE is None:
        _NC_CACHE = build_kernel()
    return _NC_CACHE


def _run(hidden_states, gate_up_proj, down_proj,
         lora_A_gu, lora_B_gu, lora_A_dn, lora_B_dn, **spmd_kwargs):
    f32 = np.float32
    hidden_states = np.ascontiguousarray(hidden_states, dtype=f32)
    gate_up_proj = np.ascontiguousarray(gate_up_proj, dtype=f32)
    down_proj = np.ascontiguousarray(down_proj, dtype=f32)
    lora_A_gu = np.ascontiguousarray(lora_A_gu, dtype=f32)
    lora_B_gu = np.ascontiguousarray(lora_B_gu, dtype=f32)
    lora_A_dn = np.ascontiguousarray(lora_A_dn, dtype=f32)
    lora_B_dn = np.ascontiguousarray(lora_B_dn, dtype=f32)

    nc = _get_nc()
    in_maps = []
    for e in range(E):
        in_maps.append({
            "x": hidden_states[T * e:T * (e + 1), :],
            "w_gu": gate_up_proj[e],
            "w_dn": down_proj[e],
            "a_gu": lora_A_gu,
            "b_gu": lora_B_gu,
            "a_dn": lora_A_dn,
            "b_dn": lora_B_dn,
        })
    res = run_bass_kernel_spmd(nc, in_maps, core_ids=list(range(E)),
                               **spmd_kwargs)
    out = np.concatenate([res.results[e]["out"] for e in range(E)], axis=0)
    return out, res


def kernel(hidden_states, gate_up_proj, down_proj,
           lora_A_gu, lora_B_gu, lora_A_dn, lora_B_dn):
    out, _ = _run(hidden_states, gate_up_proj, down_proj,
                  lora_A_gu, lora_B_gu, lora_A_dn, lora_B_dn)
    return out



# revision 2
# speedup vs baseline: 1.2612x; 1.2612x over previous
"""Llama4 MoE experts + shared LoRA, expert-parallel on 8 TRN2 NeuronCores.

Per-core (expert e): x[1024,1024] @ W_gu[1024,4096] (+ rank-8 LoRA) -> SwiGLU
-> h[1024,2048] @ W_dn[2048,1024] (+ rank-8 LoRA) -> out[1024,1024].

Layout strategy: all contractions keep the reduced dim on partitions and the
intermediate transposed. x arrives from the host already transposed ([H, T]),
as do the tiny LoRA factors, so the kernel runs zero on-chip transposes:
gate_up^T tiles [128 F-part, 512 T-free] come out of PSUM with H contracted,
SwiGLU produces hiddenT with I on partitions -- the contraction layout the
down matmul needs -- and the down output lands as natural [T, H] rows.

All matmuls run in float32r (fp32 with RNE-rounded multiply, 1 cycle/row for
free-dim >= 256 -- bf16 speed at ~1.5e-4 relative error). Weight DMAs are
laid out for >=1KiB contiguous runs per descriptor.
"""
import sys

sys.path.insert(0, "/opt/trn_rl_repo")

import numpy as np

import concourse.bacc as bacc
import concourse.bass as bass
import concourse.mybir as mybir
import concourse.tile as tile
from concourse.bass_utils import run_bass_kernel_spmd

E = 8           # experts == cores
T = 1024        # tokens per expert
H = 1024        # hidden
I = 2048        # intermediate
F2 = 2 * I      # gate+up
R = 8           # lora rank
SCALING = 2.0   # lora_alpha / rank
P = 128         # partitions
NFREE = 512     # moving free-dim per matmul (one PSUM bank of fp32)
KH = H // P     # 8 k-tiles over H
KI = I // P     # 16 k-tiles over I
NT = T // NFREE     # 2 T-chunks
FC = 256            # gate-column chunk per weight DMA
NFC = I // FC       # 8 weight-chunk iterations
JW = FC // P        # 2 f-subtiles per chunk

F32 = mybir.dt.float32
F32R = mybir.dt.float32r


def build_kernel():
    nc = bacc.Bacc("TRN2", target_bir_lowering=False, debug=False)

    xT_d = nc.dram_tensor("xT", [H, T], F32R, kind="ExternalInput")
    wgu_d = nc.dram_tensor("w_gu", [H, F2], F32R, kind="ExternalInput")
    wdn_d = nc.dram_tensor("w_dn", [I, H], F32R, kind="ExternalInput")
    aguT_d = nc.dram_tensor("a_guT", [H, R], F32R, kind="ExternalInput")
    bguT_d = nc.dram_tensor("b_guT", [R, F2], F32R, kind="ExternalInput")
    adnT_d = nc.dram_tensor("a_dnT", [I, R], F32R, kind="ExternalInput")
    bdnT_d = nc.dram_tensor("b_dnT", [R, H], F32R, kind="ExternalInput")
    out_d = nc.dram_tensor("out", [T, H], F32, kind="ExternalOutput")

    with tile.TileContext(nc) as tc:
        with (
            tc.tile_pool(name="xT", bufs=1) as xT_pool,
            tc.tile_pool(name="hT", bufs=1) as hT_pool,
            tc.tile_pool(name="smalls", bufs=1) as small_pool,
            tc.tile_pool(name="wg", bufs=2) as wg_pool,
            tc.tile_pool(name="wu", bufs=2) as wu_pool,
            tc.tile_pool(name="wdn", bufs=4) as wdn_pool,
            tc.tile_pool(name="silu", bufs=3) as silu_pool,
            tc.tile_pool(name="outs", bufs=3) as out_pool,
            tc.tile_pool(name="ps", bufs=8, space="PSUM") as ps_pool,
        ):
            # ---- resident inputs, all pre-transposed on the host ----
            xT = [xT_pool.tile([P, T], F32R, tag=f"xT{k}", name=f"xT{k}")
                  for k in range(KH)]
            for k in range(KH):
                nc.sync.dma_start(xT[k][:], xT_d[P * k:P * (k + 1), :])

            aguT = small_pool.tile([P, KH, R], F32R, tag="aguT")
            nc.sync.dma_start(aguT[:],
                              aguT_d[:].rearrange("(ko ki) r -> ki ko r", ki=P))
            adnT = small_pool.tile([P, KI, R], F32R, tag="adnT")
            nc.sync.dma_start(adnT[:],
                              adnT_d[:].rearrange("(ko ki) r -> ki ko r", ki=P))
            bguT = small_pool.tile([R, F2], F32R, tag="bguT")
            nc.sync.dma_start(bguT[:], bguT_d[:])
            bdnT = small_pool.tile([R, H], F32R, tag="bdnT")
            nc.sync.dma_start(bdnT[:], bdnT_d[:])

            # ---- r1T = SCALING * (A_gu @ x^T): [8 R, 1024 T] ----
            r1T = small_pool.tile([R, T], F32R, tag="r1T")
            for t in range(NT):
                ts = slice(NFREE * t, NFREE * (t + 1))
                ps = ps_pool.tile([R, NFREE], F32, tag="ps")
                for k in range(KH):
                    nc.tensor.matmul(ps[:], aguT[:, k, :], xT[k][:, ts],
                                     start=(k == 0), stop=(k == KH - 1))
                nc.vector.tensor_scalar_mul(r1T[:, ts], ps[:], SCALING)

            # ---- gate_up^T + SwiGLU -> hiddenT[i] [128 I, 1024 T] ----
            hT = [hT_pool.tile([P, T], F32R, tag=f"hT{k}", name=f"hT{k}")
                  for k in range(KI)]
            for fc in range(NFC):
                fg, fu = FC * fc, I + FC * fc
                wg = wg_pool.tile([P, KH, FC], F32R, tag="wg")
                wu = wu_pool.tile([P, KH, FC], F32R, tag="wu")
                nc.sync.dma_start(
                    wg[:], wgu_d[:, fg:fg + FC].rearrange(
                        "(ko ki) f -> ki ko f", ki=P))
                nc.sync.dma_start(
                    wu[:], wgu_d[:, fu:fu + FC].rearrange(
                        "(ko ki) f -> ki ko f", ki=P))
                for j in range(JW):
                    i = JW * fc + j
                    fs = slice(P * j, P * (j + 1))
                    for t in range(NT):
                        ts = slice(NFREE * t, NFREE * (t + 1))
                        psg = ps_pool.tile([P, NFREE], F32, tag="ps")
                        psu = ps_pool.tile([P, NFREE], F32, tag="ps")
                        # interleave the two accumulation chains so chain
                        # boundaries hide in each other's moving passes
                        for k in range(KH):
                            nc.tensor.matmul(psg[:], wg[:, k, fs], xT[k][:, ts],
                                             start=(k == 0), stop=False)
                            nc.tensor.matmul(psu[:], wu[:, k, fs], xT[k][:, ts],
                                             start=(k == 0), stop=False)
                        nc.tensor.matmul(psg[:], bguT[:, fg + P * j:fg + P * (j + 1)],
                                         r1T[:, ts], start=False, stop=True)
                        nc.tensor.matmul(psu[:], bguT[:, fu + P * j:fu + P * (j + 1)],
                                         r1T[:, ts], start=False, stop=True)
                        sg = silu_pool.tile([P, NFREE], F32, tag="silu")
                        nc.scalar.activation(sg[:], psg[:],
                                             mybir.ActivationFunctionType.Silu)
                        nc.vector.tensor_mul(hT[i][:, ts], sg[:], psu[:])

            # ---- r2T = SCALING * (A_dn @ hidden^T): [8 R, 1024 T] ----
            r2T = small_pool.tile([R, T], F32R, tag="r2T")
            for t in range(NT):
                ts = slice(NFREE * t, NFREE * (t + 1))
                ps = ps_pool.tile([R, NFREE], F32, tag="ps")
                for k in range(KI):
                    nc.tensor.matmul(ps[:], adnT[:, k, :], hT[k][:, ts],
                                     start=(k == 0), stop=(k == KI - 1))
                nc.vector.tensor_scalar_mul(r2T[:, ts], ps[:], SCALING)

            # ---- out[T, H] = hidden @ W_dn + lora ----
            for hg in range(NT):
                hs = slice(NFREE * hg, NFREE * (hg + 1))
                pos = [ps_pool.tile([P, NFREE], F32, tag="ps",
                                    name=f"po{hg}_{j}") for j in range(T // P)]
                for k in range(KI):
                    wd = wdn_pool.tile([P, NFREE], F32R, tag="wdn")
                    nc.sync.dma_start(wd[:], wdn_d[P * k:P * (k + 1), hs])
                    for j in range(T // P):
                        nc.tensor.matmul(pos[j][:], hT[k][:, P * j:P * (j + 1)],
                                         wd[:], start=(k == 0), stop=False)
                for j in range(T // P):
                    nc.tensor.matmul(pos[j][:], r2T[:, P * j:P * (j + 1)],
                                     bdnT[:, hs], start=False, stop=True)
                    ot = out_pool.tile([P, NFREE], F32, tag="outs")
                    nc.scalar.activation(ot[:], pos[j][:],
                                         mybir.ActivationFunctionType.Copy)
                    nc.sync.dma_start(out_d[P * j:P * (j + 1), hs], ot[:])

    nc.finalize()
    return nc


_NC_CACHE = None


def _get_nc():
    global _NC_CACHE
    if _NC_CACHE is None:
        _NC_CACHE = build_kernel()
    return _NC_CACHE


def _run(hidden_states, gate_up_proj, down_proj,
         lora_A_gu, lora_B_gu, lora_A_dn, lora_B_dn, **spmd_kwargs):
    f32 = np.float32
    hidden_states = np.asarray(hidden_states, dtype=f32)
    gate_up_proj = np.ascontiguousarray(gate_up_proj, dtype=f32)
    down_proj = np.ascontiguousarray(down_proj, dtype=f32)
    aguT = np.ascontiguousarray(np.asarray(lora_A_gu, dtype=f32).T)
    bguT = np.ascontiguousarray(np.asarray(lora_B_gu, dtype=f32).T)
    adnT = np.ascontiguousarray(np.asarray(lora_A_dn, dtype=f32).T)
    bdnT = np.ascontiguousarray(np.asarray(lora_B_dn, dtype=f32).T)

    nc = _get_nc()
    in_maps = []
    for e in range(E):
        in_maps.append({
            "xT": np.ascontiguousarray(hidden_states[T * e:T * (e + 1), :].T),
            "w_gu": gate_up_proj[e],
            "w_dn": down_proj[e],
            "a_guT": aguT,
            "b_guT": bguT,
            "a_dnT": adnT,
            "b_dnT": bdnT,
        })
    res = run_bass_kernel_spmd(nc, in_maps, core_ids=list(range(E)),
                               **spmd_kwargs)
    out = np.concatenate([res.results[e]["out"] for e in range(E)], axis=0)
    return out, res


def kernel(hidden_states, gate_up_proj, down_proj,
           lora_A_gu, lora_B_gu, lora_A_dn, lora_B_dn):
    out, _ = _run(hidden_states, gate_up_proj, down_proj,
                  lora_A_gu, lora_B_gu, lora_A_dn, lora_B_dn)
    return out
